# revision 5
# baseline (speedup 1.0000x reference)
"""Multi-head attention (b=8, n=1024, dim=1024, 16 heads x 64) on 8 TRN2 cores.

Sharding: data-parallel over batch (one batch element per core), SPMD NEFF.

Per-core dataflow (all matmuls f32r = full-speed PE with ~1e-4 matmul error):
  A) xT = PE-transpose(x);  qkT[f,t] = w_qk.T @ xT;  v[t,f] = xT.T @ w_v
     (v staged to DRAM scratch with a ones column appended per head)
  B) per head: S^T[j,i] = kT.T @ qT (K=64);  P^T = exp(0.125*S^T) on ACT;
     [outT|l] = [v|1].T @ P^T (M=65, softmax denominator for free);
     normalize with reciprocal + K=1 ones-broadcast matmul
  C) out = concat.T @ w_out + b_out (natural layout), DMA to DRAM
"""
import sys

sys.path.insert(0, "/opt/trn_rl_repo")

import numpy as np

import concourse.mybir as mybir
import concourse.tile as tile
from concourse import bacc
from concourse.bass_utils import run_bass_kernel_spmd
from concourse.masks import make_identity

FP32 = mybir.dt.float32
F32R = mybir.dt.float32r
AF = mybir.ActivationFunctionType
MUL = mybir.AluOpType.mult
ADD = mybir.AluOpType.add

N = 1024      # seq len
D = 1024      # model dim
H = 16        # heads
HD = 64       # head dim
SCALE = HD ** -0.5
NC_ = 8       # n cores = batch
KC = D // 128  # 8 contraction chunks


def build_attention_nc():
    nc = bacc.Bacc("TRN2", target_bir_lowering=False, debug=False, num_devices=1)

    x = nc.dram_tensor("x", [N, D], FP32, kind="ExternalInput").ap()
    w_qkv = nc.dram_tensor("w_qkv", [D, 3 * D], FP32, kind="ExternalInput").ap()
    w_out = nc.dram_tensor("w_out", [D, D], FP32, kind="ExternalInput").ap()
    b_out = nc.dram_tensor("b_out", [1, D], FP32, kind="ExternalInput").ap()
    out = nc.dram_tensor("out", [N, D], FP32, kind="ExternalOutput").ap()

    with tile.TileContext(nc) as tc:
        with (
            tc.tile_pool(name="persist", bufs=1) as pp,
            tc.tile_pool(name="dram", bufs=1, space="DRAM") as dp,
        ):
            # v (natural layout) staged to DRAM with a ones column per head:
            # [j, h*65+d], col 64 of each head block == 1.0
            v_dram = dp.tile([N, H * 65], F32R)

            ident = pp.tile([128, 128], FP32, tag="ident")
            make_identity(nc, ident[:])
            ones = pp.tile([1, 128], FP32, tag="ones")
            nc.vector.memset(ones[:], 1.0)

            qkT = [pp.tile([128, N], F32R, tag=f"qk{f}", name=f"qk{f}") for f in range(H)]
            cat = [pp.tile([128, N], F32R, tag=f"cat{c}", name=f"cat{c}") for c in range(KC)]

            # ---------------- Phase A: xT, qkT, v ----------------
            with (
                tc.tile_pool(name="pa", bufs=1) as pa,
                tc.tile_pool(name="pa2", bufs=2) as pa2,
                tc.tile_pool(name="psa", bufs=3, space="PSUM") as psa,
            ):
                xT = [pa.tile([128, N], F32R, tag=f"xt{c}", name=f"xt{c}") for c in range(KC)]
                stage = [pa.tile([128, H * 65], FP32, tag=f"stg{i}", name=f"stg{i}") for i in range(2)]
                nc.vector.memset(stage[0][:], 1.0)
                nc.vector.memset(stage[1][:], 1.0)

                # transpose x -> xT
                for tc_i in range(8):
                    x_sb = pa2.tile([128, D], FP32, tag="x")
                    nc.sync.dma_start(x_sb[:], x[tc_i * 128:(tc_i + 1) * 128, :])
                    for dc in range(8):
                        tr_ps = psa.tile([128, 128], FP32, tag="tr")
                        nc.tensor.transpose(
                            tr_ps[:], x_sb[:, dc * 128:(dc + 1) * 128], ident[:]
                        )
                        nc.vector.tensor_copy(
                            xT[dc][:, tc_i * 128:(tc_i + 1) * 128], tr_ps[:]
                        )

                # qkT[f, t] = w_qk[:, f].T @ xT   (stationary w column-blocks)
                for fc in range(H):
                    wc = pa2.tile([128, KC, 128], F32R, tag="wc")
                    nc.sync.dma_start(
                        wc[:],
                        w_qkv[:, fc * 128:(fc + 1) * 128]
                        .rearrange("(ko p) f -> p ko f", p=128)
                        .bitcast(F32R),
                    )
                    for ic in range(2):
                        mm = psa.tile([128, 512], FP32, tag="mm")
                        for kc in range(KC):
                            nc.tensor.matmul(
                                mm[:],
                                wc[:, kc, :],
                                xT[kc][:, ic * 512:(ic + 1) * 512],
                                start=(kc == 0),
                                stop=(kc == KC - 1),
                            )
                        nc.vector.tensor_copy(qkT[fc][:, ic * 512:(ic + 1) * 512], mm[:])

                # v[t, f] = xT.T @ w_v (w_v natural as moving operand), stage+ones -> DRAM
                wv = [pa.tile([128, KC, 512], F32R, tag=f"wv{fs}", name=f"wv{fs}") for fs in range(2)]
                for fs in range(2):
                    nc.sync.dma_start(
                        wv[fs][:],
                        w_qkv[:, 2 * D + fs * 512:2 * D + (fs + 1) * 512]
                        .rearrange("(ko p) f -> p ko f", p=128)
                        .bitcast(F32R),
                    )
                for tc_i in range(8):
                    stg = stage[tc_i % 2]
                    for fs in range(2):
                        mm = psa.tile([128, 512], FP32, tag="mm")
                        for kc in range(KC):
                            nc.tensor.matmul(
                                mm[:],
                                xT[kc][:, tc_i * 128:(tc_i + 1) * 128],
                                wv[fs][:, kc, :],
                                start=(kc == 0),
                                stop=(kc == KC - 1),
                            )
                        for hh in range(8):
                            h = fs * 8 + hh
                            nc.vector.tensor_copy(
                                stg[:, h * 65:h * 65 + 64], mm[:, hh * 64:(hh + 1) * 64]
                            )
                    nc.sync.dma_start(
                        v_dram[tc_i * 128:(tc_i + 1) * 128, :], stg[:].bitcast(F32R)
                    )

            # ---------------- Phases B+C ----------------
            with tc.tile_pool(name="pbc", bufs=1) as pbc:
                w_out_sb = [pbc.tile([128, D], F32R, tag=f"wo{c}", name=f"wo{c}") for c in range(KC)]
                for kc in range(KC):
                    nc.sync.dma_start(
                        w_out_sb[kc][:],
                        w_out[kc * 128:(kc + 1) * 128, :].bitcast(F32R),
                    )

                # Phase B: attention per head
                with (
                    tc.tile_pool(name="pb", bufs=3) as pb,
                    tc.tile_pool(name="pbp", bufs=8) as pbp,
                    tc.tile_pool(name="pb2", bufs=2) as pb2,
                    tc.tile_pool(name="psb", bufs=2, space="PSUM") as psb,
                ):
                    for h in range(H):
                        v_ext = pb.tile([128, KC, 65], F32R, tag="vx")
                        nc.sync.dma_start(
                            v_ext[:],
                            v_dram[:, h * 65:(h + 1) * 65]
                            .rearrange("(ko p) d -> p ko d", p=128),
                        )
                        qc, po = h // 2, (h % 2) * 64
                        qt, kt = qkT[qc], qkT[8 + qc]
                        for ic in range(2):
                            o_ps = psb.tile([128, 512], FP32, tag="o")
                            for jg in range(4):
                                s_ps = psb.tile([128, 2, 512], FP32, tag="s")
                                for jj in range(2):
                                    jc = jg * 2 + jj
                                    nc.tensor.matmul(
                                        s_ps[:, jj, :],
                                        kt[po:po + 64, jc * 128:(jc + 1) * 128],
                                        qt[po:po + 64, ic * 512:(ic + 1) * 512],
                                        start=True,
                                        stop=True,
                                    )
                                pt = pbp.tile([128, 2, 512], F32R, tag="pt")
                                nc.scalar.activation(
                                    pt[:].rearrange("p a b -> p (a b)"),
                                    s_ps[:].rearrange("p a b -> p (a b)"),
                                    AF.Exp,
                                    scale=SCALE,
                                )
                                for jj in range(2):
                                    jc = jg * 2 + jj
                                    nc.tensor.matmul(
                                        o_ps[0:65, :],
                                        v_ext[:, jc, :],
                                        pt[:, jj, :],
                                        start=(jc == 0),
                                        stop=(jc == KC - 1),
                                    )
                            rec = pb2.tile([1, 512], FP32, tag="rec")
                            nc.vector.reciprocal(rec[:], o_ps[64:65, :])
                            b_ps = psb.tile([64, 512], FP32, tag="b")
                            nc.tensor.matmul(
                                b_ps[:], ones[:, 0:64], rec[:], start=True, stop=True
                            )
                            bc_sb = pb2.tile([64, 512], FP32, tag="bc")
                            nc.vector.tensor_copy(bc_sb[:], b_ps[:])
                            nc.vector.tensor_tensor(
                                cat[qc][po:po + 64, ic * 512:(ic + 1) * 512],
                                o_ps[0:64, :],
                                bc_sb[:],
                                MUL,
                            )

                # Phase C: out = cat.T @ w_out + b_out
                with (
                    tc.tile_pool(name="pc", bufs=3) as pc,
                    tc.tile_pool(name="pc1", bufs=1) as pc1,
                    tc.tile_pool(name="psc", bufs=3, space="PSUM") as psc,
                    tc.tile_pool(name="psc1", bufs=1, space="PSUM") as psc1,
                ):
                    b_row = pc1.tile([1, D], FP32, tag="brow")
                    nc.sync.dma_start(b_row[:], b_out[:])
                    b_sb = pc1.tile([128, D], FP32, tag="bsb")
                    for half in range(2):
                        bb_ps = psc1.tile([128, 2, 512], FP32, tag="bb")
                        nc.tensor.matmul(
                            bb_ps[:, half, :],
                            ones[:],
                            b_row[:, half * 512:(half + 1) * 512],
                            start=True,
                            stop=True,
                        )
                        nc.vector.tensor_copy(
                            b_sb[:, half * 512:(half + 1) * 512], bb_ps[:, half, :]
                        )
                    for tc_i in range(8):
                        out_sb = pc.tile([128, D], FP32, tag="osb")
                        for mc in range(2):
                            c_ps = psc.tile([128, 512], FP32, tag="c")
                            for kc in range(KC):
                                nc.tensor.matmul(
                                    c_ps[:],
                                    cat[kc][:, tc_i * 128:(tc_i + 1) * 128],
                                    w_out_sb[kc][:, mc * 512:(mc + 1) * 512],
                                    start=(kc == 0),
                                    stop=(kc == KC - 1),
                                )
                            nc.vector.tensor_tensor(
                                out_sb[:, mc * 512:(mc + 1) * 512],
                                c_ps[:],
                                b_sb[:, mc * 512:(mc + 1) * 512],
                                ADD,
                            )
                        nc.sync.dma_start(
                            out[tc_i * 128:(tc_i + 1) * 128, :], out_sb[:]
                        )

    nc.compile()
    return nc


_NC_CACHE = None


def _get_nc():
    global _NC_CACHE
    if _NC_CACHE is None:
        _NC_CACHE = build_attention_nc()
    return _NC_CACHE


def kernel(x, w_qkv, w_out, b_out, _trace=False, **_kw):
    x = np.ascontiguousarray(x, dtype=np.float32)
    w_qkv = np.ascontiguousarray(w_qkv, dtype=np.float32)
    w_out = np.ascontiguousarray(w_out, dtype=np.float32)
    b_row = np.ascontiguousarray(b_out, dtype=np.float32).reshape(1, D)

    nc = _get_nc()
    in_maps = [
        {"x": x[b], "w_qkv": w_qkv, "w_out": w_out, "b_out": b_row}
        for b in range(NC_)
    ]
    res = run_bass_kernel_spmd(nc, in_maps, core_ids=list(range(NC_)), trace=_trace)
    out = np.stack([res.results[b]["out"] for b in range(NC_)], axis=0)
    if _trace:
        return out, res
    return out
